# revision 7
# baseline (speedup 1.0000x reference)
"""Bidirectional LSTM (TF BasicLSTMCell semantics) on 8 Trainium2 NeuronCores.

Sharding: data-parallel on (batch, direction). Cores 0-3 run the forward
LSTM over 16 batches each; cores 4-7 run the backward LSTM over the
host-reversed sequences for 16 batches each. Weights are replicated per
direction. All 8 cores run one identical Bass program; only input data
differs per core.

Per-core layout is "gate-major": features/gates live on SBUF partitions,
the 16 sequences live on the free axis. The recurrent matmul keeps W_h
stationary (bf16, fast-weight-load bound) and streams h (N=16 moving).
The input projection x @ W_x is precomputed in 16-step stripes using the
otherwise-idle PE moving port. Outputs are transposed back to
batch-major with PE transposes at stripe boundaries.

The time loop is a Tile For_i over stripe PAIRS (so xp/xts buffers
ping-pong statically inside the body); the loop back-edge resets
semaphores, which a fully-unrolled 512-step program would overflow.
DRAM tensors are padded by one/two stripes so the first/last iteration
needs no control flow: iteration 0 "flushes" garbage into a leading pad
stripe of the output, and the last iteration prefetches zero-padded
input stripes.
"""

import sys

sys.path.insert(0, "/opt/trn_rl_repo")

import numpy as np
import ml_dtypes

B, T, D, H = 64, 512, 512, 768
FORGET_BIAS = 1.0

S = 16         # sequences per core
ST = 16        # steps per stripe
KT_X = D // 128    # 4  k-tiles of the input projection
KT_H = H // 128    # 6  k-tiles of the recurrence
MT = 4 * H // 128  # 24 gate m-tiles

_PROGRAM_CACHE = {}


def _build_program(t_steps):
    import concourse.bass as bass
    import concourse.mybir as mybir
    import concourse.tile as tile
    from concourse import bacc
    from concourse.bass import ds, ts
    from concourse.masks import make_identity

    f32 = mybir.dt.float32
    bf16 = mybir.dt.bfloat16
    u8 = mybir.dt.uint8
    AF = mybir.ActivationFunctionType
    ET = mybir.EngineType

    n_stripes = t_steps // ST
    assert n_stripes * ST == t_steps and n_stripes % 2 == 0

    nc = bacc.Bacc(
        "TRN2", target_bir_lowering=False, debug=False, num_devices=8
    )

    # xts is padded with 2 zero stripes at the end (loop prefetch overrun);
    # outd has one garbage pad stripe at the START (iteration-0 flush).
    wt_d = nc.dram_tensor("wt", [128, KT_X + KT_H, 4 * H], bf16, kind="ExternalInput")
    b_d = nc.dram_tensor("bcol", [128, MT], f32, kind="ExternalInput")
    sl_d = nc.dram_tensor("slen", [128, KT_H, S], f32, kind="ExternalInput")
    xt_d = nc.dram_tensor(
        "xts", [128, KT_X, (t_steps + 2 * ST) * S], bf16, kind="ExternalInput"
    )
    out_d = nc.dram_tensor("outd", [S, t_steps + ST, H], f32, kind="ExternalOutput")
    st_d = nc.dram_tensor("stated", [S, H], f32, kind="ExternalOutput")

    wt_a, b_a, sl_a, xt_a = wt_d.ap(), b_d.ap(), sl_d.ap(), xt_d.ap()
    out_a, st_a = out_d.ap(), st_d.ap()

    with tile.TileContext(nc) as tc:
        with (
            tc.tile_pool(name="const", bufs=1) as constp,
            tc.tile_pool(name="xtsp", bufs=1) as xtsp,
            tc.tile_pool(name="xpp", bufs=1) as xpp,
            tc.tile_pool(name="outsp", bufs=1) as outsp,
            tc.tile_pool(name="osbp", bufs=2) as osbp,
            tc.tile_pool(name="workp", bufs=2) as workp,
            tc.tile_pool(name="psg", bufs=2, space=bass.MemorySpace.PSUM) as psgp,
            tc.tile_pool(name="psx", bufs=2, space=bass.MemorySpace.PSUM) as psxp,
            tc.tile_pool(name="pst", bufs=2, space=bass.MemorySpace.PSUM) as pstp,
        ):
            wt = constp.tile([128, KT_X + KT_H, 4 * H], bf16, tag="wt", name="wt_sb")
            bcol = constp.tile([128, MT], f32, tag="bcol", name="bcol_sb")
            slen = constp.tile([128, KT_H, S], f32, tag="slen", name="slen_sb")
            ident = constp.tile([128, 128], f32, tag="ident", name="ident_sb")
            h_sb = constp.tile([128, KT_H, S], f32, tag="h", name="h_sb")
            c_sb = constp.tile([128, KT_H, S], f32, tag="c", name="c_sb")
            h_bf = constp.tile([128, KT_H, S], bf16, tag="hbf", name="hbf_sb")
            t_vec = constp.tile([128, 1], f32, tag="tvec", name="tvec_sb")

            # static double buffers (ping-pong inside the For_i body)
            xtsA = xtsp.tile([128, KT_X, ST * S], bf16, tag="xtsA", name="xtsA_sb")
            xtsB = xtsp.tile([128, KT_X, ST * S], bf16, tag="xtsB", name="xtsB_sb")
            xpA = xpp.tile([128, MT, ST * S], f32, tag="xpA", name="xpA_sb")
            xpB = xpp.tile([128, MT, ST * S], f32, tag="xpB", name="xpB_sb")
            outA = outsp.tile([128, KT_H, ST, S], f32, tag="outA", name="outA_sb")
            outB = outsp.tile([128, KT_H, ST, S], f32, tag="outB", name="outB_sb")

            nc.sync.dma_start(wt[:], wt_a[:])
            nc.sync.dma_start(bcol[:], b_a[:])
            nc.sync.dma_start(slen[:], sl_a[:])
            make_identity(nc, ident[:])
            nc.gpsimd.memset(h_sb[:], 0.0)
            nc.gpsimd.memset(c_sb[:], 0.0)
            nc.gpsimd.memset(h_bf[:], 0.0)
            nc.gpsimd.memset(t_vec[:], 0.0)

            def load_xts(dst, stripe_off):
                # stripe_off: element offset expression (stripe_idx * ST * S)
                nc.sync.dma_start(dst[:], xt_a[:, :, ds(stripe_off, ST * S)])

            def xp_mgroup(m, xp_tile, xts_tile):
                # xp[:, m, :] = W_x[:, m-tile].T @ x_stripe + b[m-tile]
                psx_t = psxp.tile([128, ST * S], f32, tag="psx", name="psx_t")
                for k in range(KT_X):
                    nc.tensor.matmul(
                        psx_t[:],
                        wt[:, k, ts(m, 128)],
                        xts_tile[:, k, :],
                        start=(k == 0),
                        stop=(k == KT_X - 1),
                    )
                nc.scalar.activation(
                    xp_tile[:, m, :], psx_t[:], AF.Identity, bias=bcol[:, ds(m, 1)]
                )

            def flush_unit(out_tile, osb_t, th, kt):
                # transpose [128h, (8t,16s)] -> [(8t,16s), 128h]
                pst_t = pstp.tile([128, 128], f32, tag="pst", name="pst_t")
                nc.tensor.transpose(
                    pst_t[:], out_tile[:, kt, ds(th * 8, 8), :], ident[:]
                )
                nc.vector.tensor_copy(osb_t[:, ts(kt, 128)], pst_t[:])

            def flush_dma(osb_t, row_off):
                nc.sync.dma_start(
                    out_a[0:S, ds(row_off, 8), :].rearrange("s t h -> t s h"),
                    osb_t[:],
                )

            def stripe_body(xp_cur, xp_nxt, xts_nxt, out_tile, flush_src, flush_row):
                """One stripe of 16 recurrence steps.

                xp_cur: xp for this stripe. xp_nxt/xts_nxt: compute next
                stripe's xp in the PE bubbles. flush_src/flush_row: previous
                stripe's output tile to transpose+store (row_off expression).
                """
                nc.gpsimd.memset(out_tile[:], 0.0)
                osb = {}
                for tl in range(ST):
                    # mask_t = (t < seq_len); t_vec counts steps on gpsimd
                    mask = workp.tile([128, KT_H, S], u8, tag="mask", name="mask_t")
                    nc.vector.tensor_tensor(
                        mask[:],
                        slen[:],
                        t_vec[:].to_broadcast((128, KT_H, S)),
                        mybir.AluOpType.is_gt,
                    )
                    nc.vector.tensor_scalar_add(t_vec[:], t_vec[:], 1.0)

                    # recurrent matmuls: gates[m-tile] += W_h[k,m].T @ h[k]
                    psg_t = psgp.tile([128, MT, S], f32, tag="psg", name="psg_t")
                    for m in range(MT):
                        for k in range(KT_H):
                            nc.tensor.matmul(
                                psg_t[:, m, :],
                                wt[:, KT_X + k, ts(m, 128)],
                                h_bf[:, k, :],
                                start=(k == 0),
                                stop=(k == KT_H - 1),
                            )

                    # fill the PE bubble: next stripe's xp m-groups
                    if xp_nxt is not None:
                        for m in range(24 * tl // ST, 24 * (tl + 1) // ST):
                            xp_mgroup(m, xp_nxt, xts_nxt)

                    # fill the PE bubble: transpose-flush of the previous stripe
                    if flush_src is not None and 2 <= tl <= 14:
                        u = tl - 2
                        if u < 12:
                            th, kt = u // 6, u % 6
                            if kt == 0:
                                osb[th] = osbp.tile(
                                    [128, H], f32, tag="osb", name="osb_t"
                                )
                            flush_unit(flush_src, osb[th], th, kt)
                            if kt == 5:
                                flush_dma(osb[th], flush_row + th * 8)

                    # gates = psum + xp_t  (bias already folded into xp)
                    gt = workp.tile([128, MT, S], f32, tag="gt", name="gt_t")
                    nc.vector.tensor_add(gt[:], psg_t[:], xp_cur[:, :, ds(tl * S, S)])
                    # activations: i | j | f,o   (m-tiles 0-5 | 6-11 | 12-23)
                    ac = workp.tile([128, MT, S], f32, tag="ac", name="ac_t")
                    nc.scalar.activation(ac[:, 0:6, :], gt[:, 0:6, :], AF.Sigmoid)
                    nc.scalar.activation(ac[:, 6:12, :], gt[:, 6:12, :], AF.Tanh)
                    nc.scalar.activation(ac[:, 12:24, :], gt[:, 12:24, :], AF.Sigmoid)

                    # c = mask ? (c * f + i * j) : c
                    t1 = workp.tile([128, KT_H, S], f32, tag="t1", name="t1_t")
                    nc.vector.tensor_mul(t1[:], ac[:, 0:6, :], ac[:, 6:12, :])
                    t2 = workp.tile([128, KT_H, S], f32, tag="t2", name="t2_t")
                    nc.vector.tensor_mul(t2[:], c_sb[:], ac[:, 12:18, :])
                    nc.vector.tensor_add(t2[:], t2[:], t1[:])
                    nc.vector.copy_predicated(c_sb[:], mask[:], t2[:])

                    # h = mask ? tanh(c) * o : h ; out[t] = mask ? tanh(c) * o : 0
                    tct = workp.tile([128, KT_H, S], f32, tag="tct", name="tct_t")
                    nc.scalar.activation(tct[:], c_sb[:], AF.Tanh)
                    ht = workp.tile([128, KT_H, S], f32, tag="ht", name="ht_t")
                    nc.vector.tensor_mul(ht[:], tct[:], ac[:, 18:24, :])
                    nc.vector.copy_predicated(h_sb[:], mask[:], ht[:])
                    nc.vector.copy_predicated(out_tile[:, :, tl, :], mask[:], ht[:])
                    nc.vector.tensor_copy(h_bf[:], h_sb[:])

            # Prologue: stripes 0,1 staged; xp for stripe 0
            load_xts(xtsA, 0)
            load_xts(xtsB, ST * S)
            for m in range(MT):
                xp_mgroup(m, xpA, xtsA)

            with tc.For_i(
                0,
                n_stripes,
                2,
                hint_engines=(ET.PE, ET.DVE, ET.Activation, ET.SP, ET.Pool),
            ) as iv:
                # iv = even stripe index a; stripes (a, a+1) this iteration.
                # xtsA holds x(a) (consumed last iter) -> refill with x(a+2)
                # now; xtsB (holding x(a+1)) is still needed by stripe a's
                # xp-build, so its refill with x(a+3) is emitted after.
                load_xts(xtsA, iv * (ST * S) + 2 * ST * S)
                # stripe a: consume xpA; build xp(a+1) in xpB from xtsB;
                # flush previous iteration's outB (stripe a-1) to rows
                # (a-1+1)*ST = iv*ST (iteration 0 writes the pad stripe).
                stripe_body(xpA, xpB, xtsB, outA, outB, iv * ST)
                load_xts(xtsB, iv * (ST * S) + 3 * ST * S)
                # stripe a+1: consume xpB; build xp(a+2) in xpA from the
                # refilled xtsA; flush outA (stripe a) to rows (a+1)*ST.
                stripe_body(xpB, xpA, xtsA, outB, outA, iv * ST + ST)

            # Epilogue: flush the last stripe (n_stripes-1, in outB) + state
            for th in range(2):
                osb_t = osbp.tile([128, H], f32, tag="osb", name="osb_t")
                for kt in range(KT_H):
                    flush_unit(outB, osb_t, th, kt)
                flush_dma(osb_t, n_stripes * ST + th * 8)

            st_sb = constp.tile([S, H], f32, tag="st", name="st_sb")
            for kt in range(KT_H):
                pst_t = pstp.tile([S, 128], f32, tag="psts", name="psts_t")
                nc.tensor.transpose(pst_t[:], h_sb[:, kt, :], ident[:])
                nc.vector.tensor_copy(st_sb[:, ts(kt, 128)], pst_t[:])
            nc.sync.dma_start(st_a[:], st_sb[:])

    nc.compile()
    return nc


def _get_program(t_steps):
    if t_steps not in _PROGRAM_CACHE:
        _PROGRAM_CACHE[t_steps] = _build_program(t_steps)
    return _PROGRAM_CACHE[t_steps]


def _reverse_sequence_np(x, seq_len):
    # mirrors tf.reverse_sequence along axis 1
    t = np.arange(x.shape[1])
    idx = np.where(
        t[None, :] < seq_len[:, None], seq_len[:, None] - 1 - t[None, :], t[None, :]
    )
    return np.take_along_axis(x, idx[:, :, None], axis=1)


def _core_inputs(x_blk, w_np, b_np, sl_blk, t_steps):
    """Build one core's input map from a [16, T, D] fp32 block."""
    bf = ml_dtypes.bfloat16
    # xts[p, k, t*16+s] = x[s, t, 128k+p]; pad 2 stripes of zeros in t
    xts = np.zeros((128, KT_X, (t_steps + 2 * ST) * S), dtype=bf)
    xts[:, :, : t_steps * S] = (
        x_blk.transpose(2, 1, 0).reshape(KT_X, 128, t_steps * S).transpose(1, 0, 2)
    ).astype(bf)
    # wt[p, k, g] = W[128k+p, g]
    wt = np.ascontiguousarray(
        w_np.reshape(KT_X + KT_H, 128, 4 * H).transpose(1, 0, 2)
    ).astype(bf)
    b_eff = b_np.astype(np.float64).copy()
    b_eff[2 * H : 3 * H] += FORGET_BIAS
    bcol = np.ascontiguousarray(b_eff.reshape(MT, 128).T).astype(np.float32)
    slen = np.broadcast_to(
        sl_blk.astype(np.float32)[None, None, :], (128, KT_H, S)
    ).copy()
    return {"wt": wt, "bcol": bcol, "slen": slen, "xts": xts}


LAST_EXEC_NS = None
LAST_RESULTS = None


def kernel(seq, seq_len, W_fw, b_fw, W_bw, b_bw, _trace=False):
    global LAST_EXEC_NS, LAST_RESULTS
    from concourse.bass_utils import run_bass_kernel_spmd

    seq = np.asarray(seq, dtype=np.float32)
    seq_len = np.asarray(seq_len, dtype=np.int32)
    W_fw = np.asarray(W_fw, dtype=np.float32)
    b_fw = np.asarray(b_fw, dtype=np.float32)
    W_bw = np.asarray(W_bw, dtype=np.float32)
    b_bw = np.asarray(b_bw, dtype=np.float32)

    b_sz, t_steps, _ = seq.shape
    assert b_sz == B and seq.shape[2] == D

    nc = _get_program(t_steps)

    seq_rev = _reverse_sequence_np(seq, seq_len)

    in_maps = []
    for c in range(8):
        blk = slice(16 * (c % 4), 16 * (c % 4) + S)
        if c < 4:
            in_maps.append(_core_inputs(seq[blk], W_fw, b_fw, seq_len[blk], t_steps))
        else:
            in_maps.append(
                _core_inputs(seq_rev[blk], W_bw, b_bw, seq_len[blk], t_steps)
            )

    res = run_bass_kernel_spmd(nc, in_maps, core_ids=list(range(8)), trace=_trace)
    LAST_EXEC_NS = res.exec_time_ns
    LAST_RESULTS = res

    outputs = np.zeros((B, t_steps, 2 * H), dtype=np.float32)
    state_h = np.zeros((B, 2 * H), dtype=np.float32)
    for c in range(8):
        blk = slice(16 * (c % 4), 16 * (c % 4) + S)
        o = res.results[c]["outd"][:, ST:, :]
        if c < 4:
            outputs[blk, :, :H] = o
            state_h[blk, :H] = res.results[c]["stated"]
        else:
            outputs[blk, :, H:] = _reverse_sequence_np(o, seq_len[blk])
            state_h[blk, H:] = res.results[c]["stated"]
    return outputs, state_h


# revision 16
# speedup vs baseline: 1.1714x; 1.1714x over previous
"""Bidirectional LSTM (TF BasicLSTMCell semantics) on 8 Trainium2 NeuronCores.

Sharding: data-parallel on (batch, direction). Cores 0-3 run the forward
LSTM over 16 batches each; cores 4-7 run the backward LSTM over the
host-reversed sequences for 16 batches each. Weights are replicated per
direction. All 8 cores run one identical Bass program; only input data
differs per core.

Per-core layout is "gate-major": features/gates live on SBUF partitions,
the 16 sequences live on the free axis. The recurrent matmul keeps W_h
stationary (bf16, fast-weight-load bound) and streams h (N=16 moving).
The input projection x @ W_x is precomputed in 16-step stripes using the
otherwise-idle PE moving port. Outputs are transposed back to
batch-major with PE transposes at stripe boundaries.

The time loop is a Tile For_i over stripe PAIRS (so xp/xts buffers
ping-pong statically inside the body); the loop back-edge resets
semaphores, which a fully-unrolled 512-step program would overflow.
DRAM tensors are padded by one/two stripes so the first/last iteration
needs no control flow: iteration 0 "flushes" garbage into a leading pad
stripe of the output, and the last iteration prefetches zero-padded
input stripes.
"""

import sys

sys.path.insert(0, "/opt/trn_rl_repo")

import numpy as np
import ml_dtypes

B, T, D, H = 64, 512, 512, 768
FORGET_BIAS = 1.0

S = 16         # sequences per core
ST = 16        # steps per stripe
KT_X = D // 128    # 4  k-tiles of the input projection
KT_H = H // 128    # 6  k-tiles of the recurrence
MT = 4 * H // 128  # 24 gate m-tiles

_PROGRAM_CACHE = {}


def _build_program(t_steps):
    import concourse.bass as bass
    import concourse.mybir as mybir
    import concourse.tile as tile
    from concourse import bacc
    from concourse.bass import ds, ts
    from concourse.masks import make_identity

    f32 = mybir.dt.float32
    bf16 = mybir.dt.bfloat16
    u8 = mybir.dt.uint8
    AF = mybir.ActivationFunctionType
    ET = mybir.EngineType

    n_stripes = t_steps // ST
    assert n_stripes * ST == t_steps and n_stripes % 2 == 0

    nc = bacc.Bacc(
        "TRN2", target_bir_lowering=False, debug=False, num_devices=8
    )

    # xts is padded with 2 zero stripes at the end (loop prefetch overrun);
    # outd has one garbage pad stripe at the START (iteration-0 flush).
    wt_d = nc.dram_tensor("wt", [128, KT_X + KT_H, 4 * H], bf16, kind="ExternalInput")
    b_d = nc.dram_tensor("bcol", [128, MT], f32, kind="ExternalInput")
    sl_d = nc.dram_tensor("slen", [128, KT_H, S], f32, kind="ExternalInput")
    xt_d = nc.dram_tensor(
        "xts", [128, KT_X, (t_steps + 2 * ST) * S], bf16, kind="ExternalInput"
    )
    out_d = nc.dram_tensor("outd", [S, t_steps + ST, H], f32, kind="ExternalOutput")
    st_d = nc.dram_tensor("stated", [S, H], f32, kind="ExternalOutput")

    wt_a, b_a, sl_a, xt_a = wt_d.ap(), b_d.ap(), sl_d.ap(), xt_d.ap()
    out_a, st_a = out_d.ap(), st_d.ap()

    with tile.TileContext(nc) as tc:
        with (
            tc.tile_pool(name="const", bufs=1) as constp,
            tc.tile_pool(name="xtsp", bufs=1) as xtsp,
            tc.tile_pool(name="xpp", bufs=1) as xpp,
            tc.tile_pool(name="outsp", bufs=1) as outsp,
            tc.tile_pool(name="osbp", bufs=2) as osbp,
            tc.tile_pool(name="workp", bufs=2) as workp,
            tc.tile_pool(name="psg", bufs=2, space=bass.MemorySpace.PSUM) as psgp,
            tc.tile_pool(name="psx", bufs=2, space=bass.MemorySpace.PSUM) as psxp,
            tc.tile_pool(name="pst", bufs=2, space=bass.MemorySpace.PSUM) as pstp,
        ):
            wt = constp.tile([128, KT_X + KT_H, 4 * H], bf16, tag="wt", name="wt_sb")
            bcol = constp.tile([128, MT], f32, tag="bcol", name="bcol_sb")
            slen = constp.tile([128, KT_H, S], f32, tag="slen", name="slen_sb")
            ident = constp.tile([128, 128], f32, tag="ident", name="ident_sb")
            h_sb = constp.tile([128, KT_H, S], f32, tag="h", name="h_sb")
            c_sb = constp.tile([128, KT_H, S], f32, tag="c", name="c_sb")
            h_bf = constp.tile([128, KT_H, S], bf16, tag="hbf", name="hbf_sb")
            t_vec = constp.tile([128, 1], f32, tag="tvec", name="tvec_sb")

            # static double buffers (ping-pong inside the For_i body)
            xtsA = xtsp.tile([128, KT_X, ST * S], bf16, tag="xtsA", name="xtsA_sb")
            xtsB = xtsp.tile([128, KT_X, ST * S], bf16, tag="xtsB", name="xtsB_sb")
            # gates layout: [128, ktile(6), gate(4)*S] so k-half slices stay
            # 2-D in the free dims (TensorTensor APs allow at most 2)
            xpA = xpp.tile([128, KT_H, ST, 4 * S], f32, tag="xpA", name="xpA_sb")
            xpB = xpp.tile([128, KT_H, ST, 4 * S], f32, tag="xpB", name="xpB_sb")
            outA = outsp.tile([128, KT_H, ST, S], f32, tag="outA", name="outA_sb")
            outB = outsp.tile([128, KT_H, ST, S], f32, tag="outB", name="outB_sb")

            nc.sync.dma_start(wt[:], wt_a[:])
            nc.sync.dma_start(bcol[:], b_a[:])
            nc.sync.dma_start(slen[:], sl_a[:])
            make_identity(nc, ident[:])
            nc.gpsimd.memset(h_sb[:], 0.0)
            nc.gpsimd.memset(c_sb[:], 0.0)
            nc.gpsimd.memset(h_bf[:], 0.0)
            nc.gpsimd.memset(t_vec[:], 0.0)

            def load_xts(dst, stripe_off):
                # stripe_off: element offset expression (stripe_idx * ST * S)
                nc.sync.dma_start(dst[:], xt_a[:, :, ds(stripe_off, ST * S)])

            def xp_mgroup(m, xp_tile, xts_tile):
                # xp[:, g, kt, :] = W_x[:, m-tile].T @ x_stripe + b[m-tile]
                psx_t = psxp.tile([128, ST * S], f32, tag="psx", name="psx_t")
                for k in range(KT_X):
                    nc.tensor.matmul(
                        psx_t[:],
                        wt[:, k, ts(m, 128)],
                        xts_tile[:, k, :],
                        start=(k == 0),
                        stop=(k == KT_X - 1),
                    )
                g, kt = divmod(m, KT_H)
                nc.scalar.activation(
                    xp_tile[:, kt, :, ts(g, S)],
                    psx_t[:].rearrange("p (t s) -> p t s", s=S),
                    AF.Identity,
                    bias=bcol[:, ds(m, 1)],
                )

            def flush_unit(out_tile, osb_t, th, kt):
                # transpose [128h, (8t,16s)] -> [(8t,16s), 128h]
                pst_t = pstp.tile([128, 128], f32, tag="pst", name="pst_t")
                nc.tensor.transpose(
                    pst_t[:], out_tile[:, kt, ds(th * 8, 8), :], ident[:]
                )
                nc.vector.tensor_copy(osb_t[:, ts(kt, 128)], pst_t[:])

            def flush_dma(osb_t, row_off):
                nc.sync.dma_start(
                    out_a[0:S, ds(row_off, 8), :].rearrange("s t h -> t s h"),
                    osb_t[:],
                )

            def stripe_body(xp_cur, xp_nxt, xts_nxt, out_tile, flush_src, flush_row):
                """One stripe of 16 recurrence steps.

                xp_cur: xp for this stripe. xp_nxt/xts_nxt: compute next
                stripe's xp in the PE bubbles. flush_src/flush_row: previous
                stripe's output tile to transpose+store (row_off expression).
                """
                nc.gpsimd.memset(out_tile[:], 0.0)
                osb = {}
                for tl in range(ST):
                    # mask_t = (t < seq_len); t_vec counts steps on gpsimd
                    mask = workp.tile([128, KT_H, S], u8, tag="mask", name="mask_t")
                    nc.vector.tensor_tensor(
                        mask[:],
                        slen[:],
                        t_vec[:].to_broadcast((128, KT_H, S)),
                        mybir.AluOpType.is_gt,
                    )
                    nc.vector.tensor_scalar_add(t_vec[:], t_vec[:], 1.0)

                    # recurrent matmuls, two contraction passes so the next
                    # step's pass A only waits on half 0 of the new h.
                    # Separate PSUM tiles: a start=True clears has_written
                    # bits bank-wide, so passes must not share a bank.
                    psgA_t = psgp.tile(
                        [128, KT_H, 4 * S], f32, tag="psgA", name="psgA_t"
                    )
                    psgB_t = psgp.tile(
                        [128, KT_H, 4 * S], f32, tag="psgB", name="psgB_t"
                    )
                    for m in range(MT):
                        g, kt = divmod(m, KT_H)
                        for k in range(3):
                            nc.tensor.matmul(
                                psgA_t[:, kt, ts(g, S)],
                                wt[:, KT_X + k, ts(m, 128)],
                                h_bf[:, k, :],
                                start=(k == 0),
                                stop=(k == 2),
                            )
                    for m in range(MT):
                        g, kt = divmod(m, KT_H)
                        for k in range(3, KT_H):
                            nc.tensor.matmul(
                                psgB_t[:, kt, ts(g, S)],
                                wt[:, KT_X + k, ts(m, 128)],
                                h_bf[:, k, :],
                                start=(k == 3),
                                stop=(k == KT_H - 1),
                            )

                    # fill the PE bubble: next stripe's xp m-groups
                    if xp_nxt is not None:
                        for m in range(24 * tl // ST, 24 * (tl + 1) // ST):
                            xp_mgroup(m, xp_nxt, xts_nxt)

                    # fill the PE bubble: transpose-flush of the previous stripe
                    if flush_src is not None and 2 <= tl <= 14:
                        u = tl - 2
                        if u < 12:
                            th, kt = u // 6, u % 6
                            if kt == 0:
                                osb[th] = osbp.tile(
                                    [128, H], f32, tag="osb", name="osb_t"
                                )
                            flush_unit(flush_src, osb[th], th, kt)
                            if kt == 5:
                                flush_dma(osb[th], flush_row + th * 8)

                    # elementwise chain, split into two k-halves so h_bf's
                    # first half is ready as early as possible
                    gt = workp.tile([128, KT_H, 4 * S], f32, tag="gt", name="gt_t")
                    ac = workp.tile([128, KT_H, 4 * S], f32, tag="ac", name="ac_t")
                    t1 = workp.tile([128, KT_H, S], f32, tag="t1", name="t1_t")
                    t2 = workp.tile([128, KT_H, S], f32, tag="t2", name="t2_t")
                    tct = workp.tile([128, KT_H, S], f32, tag="tct", name="tct_t")
                    ht = workp.tile([128, KT_H, S], f32, tag="ht", name="ht_t")
                    for hx in range(2):
                        kr = slice(3 * hx, 3 * hx + 3)
                        # gates = psumA + psumB + xp (bias folded into xp);
                        # each add reads at most one PSUM operand
                        nc.vector.tensor_add(
                            gt[:, kr, :], psgA_t[:, kr, :], xp_cur[:, kr, tl, :]
                        )
                        nc.vector.tensor_add(
                            gt[:, kr, :], psgB_t[:, kr, :], gt[:, kr, :]
                        )
                        nc.scalar.activation(
                            ac[:, kr, ts(0, S)], gt[:, kr, ts(0, S)], AF.Sigmoid
                        )
                        nc.scalar.activation(
                            ac[:, kr, ts(1, S)], gt[:, kr, ts(1, S)], AF.Tanh
                        )
                        nc.scalar.activation(
                            ac[:, kr, ts(2, S)], gt[:, kr, ts(2, S)], AF.Sigmoid
                        )
                        nc.scalar.activation(
                            ac[:, kr, ts(3, S)], gt[:, kr, ts(3, S)], AF.Sigmoid
                        )
                        # c = mask ? (c * f + i * j) : c
                        nc.vector.tensor_mul(
                            t1[:, kr, :], ac[:, kr, ts(0, S)], ac[:, kr, ts(1, S)]
                        )
                        nc.vector.tensor_mul(
                            t2[:, kr, :], c_sb[:, kr, :], ac[:, kr, ts(2, S)]
                        )
                        nc.vector.tensor_add(t2[:, kr, :], t2[:, kr, :], t1[:, kr, :])
                        nc.vector.copy_predicated(
                            c_sb[:, kr, :], mask[:, kr, :], t2[:, kr, :]
                        )
                        # h = mask ? tanh(c) * o : h (h_bf updated directly)
                        nc.scalar.activation(tct[:, kr, :], c_sb[:, kr, :], AF.Tanh)
                        nc.vector.tensor_mul(
                            ht[:, kr, :], tct[:, kr, :], ac[:, kr, ts(3, S)]
                        )
                        nc.vector.copy_predicated(
                            h_bf[:, kr, :], mask[:, kr, :], ht[:, kr, :]
                        )
                        nc.vector.copy_predicated(
                            h_sb[:, kr, :], mask[:, kr, :], ht[:, kr, :]
                        )
                        nc.vector.copy_predicated(
                            out_tile[:, kr, tl, :], mask[:, kr, :], ht[:, kr, :]
                        )

            # Prologue: stripes 0,1 staged; xp for stripe 0
            load_xts(xtsA, 0)
            load_xts(xtsB, ST * S)
            for m in range(MT):
                xp_mgroup(m, xpA, xtsA)

            with tc.For_i(
                0,
                n_stripes,
                2,
                hint_engines=(ET.PE, ET.DVE, ET.Activation, ET.SP, ET.Pool),
            ) as iv:
                # iv = even stripe index a; stripes (a, a+1) this iteration.
                # xtsA holds x(a) (consumed last iter) -> refill with x(a+2)
                # now; xtsB (holding x(a+1)) is still needed by stripe a's
                # xp-build, so its refill with x(a+3) is emitted after.
                load_xts(xtsA, iv * (ST * S) + 2 * ST * S)
                # stripe a: consume xpA; build xp(a+1) in xpB from xtsB;
                # flush previous iteration's outB (stripe a-1) to rows
                # (a-1+1)*ST = iv*ST (iteration 0 writes the pad stripe).
                stripe_body(xpA, xpB, xtsB, outA, outB, iv * ST)
                load_xts(xtsB, iv * (ST * S) + 3 * ST * S)
                # stripe a+1: consume xpB; build xp(a+2) in xpA from the
                # refilled xtsA; flush outA (stripe a) to rows (a+1)*ST.
                stripe_body(xpB, xpA, xtsA, outB, outA, iv * ST + ST)

            # Epilogue: flush the last stripe (n_stripes-1, in outB) + state
            for th in range(2):
                osb_t = osbp.tile([128, H], f32, tag="osb", name="osb_t")
                for kt in range(KT_H):
                    flush_unit(outB, osb_t, th, kt)
                flush_dma(osb_t, n_stripes * ST + th * 8)

            st_sb = constp.tile([S, H], f32, tag="st", name="st_sb")
            for kt in range(KT_H):
                pst_t = pstp.tile([S, 128], f32, tag="pst", name="psts_t")
                nc.tensor.transpose(pst_t[:], h_sb[:, kt, :], ident[:])
                nc.vector.tensor_copy(st_sb[:, ts(kt, 128)], pst_t[:])
            nc.sync.dma_start(st_a[:], st_sb[:])

    nc.compile()
    return nc


def _get_program(t_steps):
    if t_steps not in _PROGRAM_CACHE:
        _PROGRAM_CACHE[t_steps] = _build_program(t_steps)
    return _PROGRAM_CACHE[t_steps]


def _reverse_sequence_np(x, seq_len):
    # mirrors tf.reverse_sequence along axis 1
    t = np.arange(x.shape[1])
    idx = np.where(
        t[None, :] < seq_len[:, None], seq_len[:, None] - 1 - t[None, :], t[None, :]
    )
    return np.take_along_axis(x, idx[:, :, None], axis=1)


def _core_inputs(x_blk, w_np, b_np, sl_blk, t_steps):
    """Build one core's input map from a [16, T, D] fp32 block."""
    bf = ml_dtypes.bfloat16
    # xts[p, k, t*16+s] = x[s, t, 128k+p]; pad 2 stripes of zeros in t
    xts = np.zeros((128, KT_X, (t_steps + 2 * ST) * S), dtype=bf)
    xts[:, :, : t_steps * S] = (
        x_blk.transpose(2, 1, 0).reshape(KT_X, 128, t_steps * S).transpose(1, 0, 2)
    ).astype(bf)
    # wt[p, k, g] = W[128k+p, g]
    wt = np.ascontiguousarray(
        w_np.reshape(KT_X + KT_H, 128, 4 * H).transpose(1, 0, 2)
    ).astype(bf)
    b_eff = b_np.astype(np.float64).copy()
    b_eff[2 * H : 3 * H] += FORGET_BIAS
    bcol = np.ascontiguousarray(b_eff.reshape(MT, 128).T).astype(np.float32)
    slen = np.broadcast_to(
        sl_blk.astype(np.float32)[None, None, :], (128, KT_H, S)
    ).copy()
    return {"wt": wt, "bcol": bcol, "slen": slen, "xts": xts}


LAST_EXEC_NS = None
LAST_RESULTS = None


def kernel(seq, seq_len, W_fw, b_fw, W_bw, b_bw, _trace=False):
    global LAST_EXEC_NS, LAST_RESULTS
    from concourse.bass_utils import run_bass_kernel_spmd

    seq = np.asarray(seq, dtype=np.float32)
    seq_len = np.asarray(seq_len, dtype=np.int32)
    W_fw = np.asarray(W_fw, dtype=np.float32)
    b_fw = np.asarray(b_fw, dtype=np.float32)
    W_bw = np.asarray(W_bw, dtype=np.float32)
    b_bw = np.asarray(b_bw, dtype=np.float32)

    b_sz, t_steps, _ = seq.shape
    assert b_sz == B and seq.shape[2] == D

    nc = _get_program(t_steps)

    seq_rev = _reverse_sequence_np(seq, seq_len)

    in_maps = []
    for c in range(8):
        blk = slice(16 * (c % 4), 16 * (c % 4) + S)
        if c < 4:
            in_maps.append(_core_inputs(seq[blk], W_fw, b_fw, seq_len[blk], t_steps))
        else:
            in_maps.append(
                _core_inputs(seq_rev[blk], W_bw, b_bw, seq_len[blk], t_steps)
            )

    res = run_bass_kernel_spmd(nc, in_maps, core_ids=list(range(8)), trace=_trace)
    LAST_EXEC_NS = res.exec_time_ns
    LAST_RESULTS = res

    outputs = np.zeros((B, t_steps, 2 * H), dtype=np.float32)
    state_h = np.zeros((B, 2 * H), dtype=np.float32)
    for c in range(8):
        blk = slice(16 * (c % 4), 16 * (c % 4) + S)
        o = res.results[c]["outd"][:, ST:, :]
        if c < 4:
            outputs[blk, :, :H] = o
            state_h[blk, :H] = res.results[c]["stated"]
        else:
            outputs[blk, :, H:] = _reverse_sequence_np(o, seq_len[blk])
            state_h[blk, H:] = res.results[c]["stated"]
    return outputs, state_h
